# revision 2
# baseline (speedup 1.0000x reference)
"""Trainium2 Bass kernel for gather + segment-sum (GNN sum-aggregator).

    out[s, :] = sum_{e : seg_ids[e] == s} features[neigh_idx[e], :]

Strategy (8 NeuronCores, SPMD single NEFF):
  - Shard the segment (destination-node) axis: core c owns segments
    [12500c, 12500(c+1)) and the contiguous slice of the sorted edge list
    that targets them. The feature table is replicated.
  - Features are split hi/lo into two bf16 halves packed side by side
    ([N, 128] bf16), so one 256B-row gather fetches an exactly-representable
    fp32 row as two bf16 matmul operands (rel err ~1e-7 after the PSUM-fp32
    accumulate).
  - Edges are grouped into 128-segment windows. Within a window they are
    sorted by neigh_idx and split into <=4 buckets of 32768 table rows so the
    optimized int16-index SWDGE dma_gather can fetch them; the 4 gathers of a
    window round-robin over 4 SWDGE queues (4x descriptor-gen parallelism,
    ~290 GB/s measured vs ~34 GB/s on one queue).
  - Segment-sum inside a window = matmul with a per-block one-hot matrix
    (lhsT = onehot[128 edges, 128 segs] bf16, built on DVE via
    tensor_scalar(is_equal) against an iota row; rhs = gathered [128, 128]
    bf16), accumulated over the window's blocks in PSUM, then flushed as one
    contiguous 128-row store. Pad slots gather row 0 and carry a one-hot
    miss value, so they contribute exactly zero.
"""

import math

import numpy as np
import ml_dtypes

N_NODES = 100000
N_EDGES = 1600000
D = 64
N_CORES = 8
SEGS_PER_CORE = N_NODES // N_CORES  # 12500
W = 128  # segments per window
NWIN = math.ceil(SEGS_PER_CORE / W)  # 98
SEG_PAD = NWIN * W  # 12544
BUCKET = 32768
NBUK = 4
MISS = 30000.0


def _wrap_idxs(idx_flat: np.ndarray) -> np.ndarray:
    """[NI] -> [128, NI//16] int16 (16-partition wrap, replicated 8x)."""
    ni = idx_flat.shape[0]
    w = idx_flat.reshape(ni // 16, 16).T.astype(np.int16)
    return np.tile(w, (8, 1))


def _prep_core(neigh: np.ndarray, seg: np.ndarray, e0: int, e1: int, seg_base: int,
               caps: list[int]):
    """Build idx16/rel arrays for one core given its edge slice [e0, e1)."""
    spw = sum(caps)
    nblk = spw // 128
    idx16 = np.zeros((128, NWIN * (spw // 16)), np.int16)
    rel = np.full((128, NWIN * nblk), MISS, np.float32)
    nidx = neigh[e0:e1]
    nseg = seg[e0:e1]
    # window of each edge (local)
    win_bounds = np.searchsorted(nseg, seg_base + np.arange(NWIN + 1) * W)
    for w in range(NWIN):
        a, b = win_bounds[w], win_bounds[w + 1]
        widx = nidx[a:b]
        wrel = (nseg[a:b] - (seg_base + w * W)).astype(np.float32)
        order = np.argsort(widx, kind="stable")
        widx = widx[order]
        wrel = wrel[order]
        bb = np.searchsorted(widx, np.arange(1, NBUK) * BUCKET)
        runs = np.split(np.arange(len(widx)), bb)
        scol = w * (spw // 16)
        sblk = w * nblk
        for k in range(NBUK):
            cap = caps[k]
            r = runs[k]
            assert len(r) <= cap, f"bucket overflow {len(r)} > {cap}"
            idxp = np.zeros(cap, np.int64)
            idxp[: len(r)] = widx[r] - k * BUCKET
            relp = np.full(cap, MISS, np.float32)
            relp[: len(r)] = wrel[r]
            idx16[:, scol : scol + cap // 16] = _wrap_idxs(idxp)
            rel[:, sblk : sblk + cap // 128] = relp.reshape(cap // 128, 128).T
            scol += cap // 16
            sblk += cap // 128
    return idx16, rel


def _compute_caps(neigh: np.ndarray, seg: np.ndarray, ebounds: np.ndarray):
    """Global per-bucket slot capacities (multiples of 128) over all (core, window)."""
    maxes = [0] * NBUK
    for c in range(N_CORES):
        e0, e1 = ebounds[c], ebounds[c + 1]
        nseg = seg[e0:e1]
        nidx = neigh[e0:e1]
        seg_base = c * SEGS_PER_CORE
        win_bounds = np.searchsorted(nseg, seg_base + np.arange(NWIN + 1) * W)
        for w in range(NWIN):
            a, b = win_bounds[w], win_bounds[w + 1]
            widx = np.sort(nidx[a:b])
            bb = np.searchsorted(widx, np.arange(NBUK + 1) * BUCKET)
            for k in range(NBUK):
                maxes[k] = max(maxes[k], bb[k + 1] - bb[k])
    return [max(128, 128 * math.ceil(m / 128)) for m in maxes]


def _build_nc(caps: list[int]):
    import concourse.bass as bass
    import concourse.tile as tile
    from concourse import bacc, mybir

    spw = sum(caps)
    nblk = spw // 128
    tab_sizes = [min(BUCKET, N_NODES - k * BUCKET) for k in range(NBUK)]

    nc = bacc.Bacc("TRN2", target_bir_lowering=False, num_swdge_queues=4)
    tabs = [
        nc.dram_tensor(f"tab{k}", [tab_sizes[k], 2 * D], mybir.dt.bfloat16,
                       kind="ExternalInput")
        for k in range(NBUK)
    ]
    idx_d = nc.dram_tensor("idx16", [128, NWIN * (spw // 16)], mybir.dt.int16,
                           kind="ExternalInput")
    rel_d = nc.dram_tensor("rel", [128, NWIN * nblk], mybir.dt.float32,
                           kind="ExternalInput")
    iota_d = nc.dram_tensor("iota", [128, W], mybir.dt.bfloat16, kind="ExternalInput")
    out_d = nc.dram_tensor("out", [SEG_PAD, D], mybir.dt.float32, kind="ExternalOutput")

    with tile.TileContext(nc) as tc:
        with (
            tc.tile_pool(name="const", bufs=1) as cpool,
            tc.tile_pool(name="g", bufs=3) as gpool,
            tc.tile_pool(name="oh", bufs=8) as ohpool,
            tc.tile_pool(name="psum", bufs=4, space="PSUM") as ppool,
            tc.tile_pool(name="flush", bufs=4) as fpool,
        ):
            idx_t = cpool.tile([128, NWIN * (spw // 16)], mybir.dt.int16)
            nc.sync.dma_start(idx_t[:], idx_d[:])
            rel_t = cpool.tile([128, NWIN * nblk], mybir.dt.float32)
            nc.sync.dma_start(rel_t[:], rel_d[:])
            iota_t = cpool.tile([128, W], mybir.dt.bfloat16)
            nc.sync.dma_start(iota_t[:], iota_d[:])

            for w in range(NWIN):
                g_t = gpool.tile([128, nblk, 2 * D], mybir.dt.bfloat16)
                scol = w * (spw // 16)
                sslot = 0
                for k in range(NBUK):
                    cap = caps[k]
                    nc.gpsimd.dma_gather(
                        g_t[:, sslot : sslot + cap // 128, :],
                        tabs[k][:],
                        idx_t[:, scol : scol + cap // 16],
                        cap, cap, 2 * D,
                        single_packet=False,
                        queue_num=k % 4,
                    )
                    scol += cap // 16
                    sslot += cap // 128
                psum_t = ppool.tile([W, 2 * D], mybir.dt.float32, space="PSUM")
                for b in range(nblk):
                    oh = ohpool.tile([128, W], mybir.dt.bfloat16)
                    nc.vector.tensor_scalar(
                        out=oh[:], in0=iota_t[:],
                        scalar1=rel_t[:, w * nblk + b : w * nblk + b + 1],
                        scalar2=None, op0=mybir.AluOpType.is_equal,
                    )
                    nc.tensor.matmul(
                        psum_t[:], lhsT=oh[:], rhs=g_t[:, b, :],
                        start=(b == 0), stop=(b == nblk - 1),
                    )
                cop = fpool.tile([W, 2 * D], mybir.dt.float32)
                nc.scalar.copy(cop[:], psum_t[:])
                comb = fpool.tile([W, D], mybir.dt.float32)
                nc.vector.tensor_tensor(
                    out=comb[:], in0=cop[:, :D], in1=cop[:, D:],
                    op=mybir.AluOpType.add,
                )
                nc.sync.dma_start(out_d[w * W : (w + 1) * W, :], comb[:])
    nc.finalize()
    return nc


class _SpmdRunner:
    """Compile once, execute the bass kernel across n_cores via PJRT shard_map."""

    def __init__(self, nc, n_cores: int):
        import jax
        import numpy as np
        from jax.experimental.shard_map import shard_map
        from jax.sharding import Mesh, NamedSharding, PartitionSpec
        import concourse.mybir as mybir
        from concourse.bass2jax import (
            _bass_exec_p, install_neuronx_cc_hook, partition_id_tensor,
        )

        install_neuronx_cc_hook()
        self.jax = jax
        self.n_cores = n_cores
        in_names, out_names, out_avals, zero_outs = [], [], [], []
        partition_name = nc.partition_id_tensor.name if nc.partition_id_tensor else None
        for alloc in nc.m.functions[0].allocations:
            if not isinstance(alloc, mybir.MemoryLocationSet):
                continue
            name = alloc.memorylocations[0].name
            if alloc.kind == "ExternalInput":
                if name != partition_name:
                    in_names.append(name)
            elif alloc.kind == "ExternalOutput":
                shape = tuple(alloc.tensor_shape)
                dtype = mybir.dt.np(alloc.dtype)
                out_names.append(name)
                out_avals.append(jax.core.ShapedArray(shape, dtype))
                zero_outs.append(np.zeros(shape, dtype))
        self.n_params = len(in_names)
        self.in_names = list(in_names)
        self.out_names = out_names
        self.out_avals = out_avals
        self.zero_outs = zero_outs
        all_in = in_names + out_names + ([partition_name] if partition_name else [])

        def _body(*args):
            operands = list(args)
            if partition_name is not None:
                operands.append(partition_id_tensor())
            outs = _bass_exec_p.bind(
                *operands,
                out_avals=tuple(out_avals),
                in_names=tuple(all_in),
                out_names=tuple(out_names),
                lowering_input_output_aliases=(),
                sim_require_finite=True,
                sim_require_nnan=True,
                nc=nc,
            )
            return tuple(outs)

        donate = tuple(range(self.n_params, self.n_params + len(out_names)))
        devices = jax.devices()[:n_cores]
        assert len(devices) >= n_cores, f"need {n_cores} cores, got {len(devices)}"
        self.mesh = Mesh(np.asarray(devices), ("core",))
        in_specs = (PartitionSpec("core"),) * (self.n_params + len(out_names))
        out_specs = (PartitionSpec("core"),) * len(out_names)
        self.fn = jax.jit(
            shard_map(_body, mesh=self.mesh, in_specs=in_specs, out_specs=out_specs,
                      check_rep=False),
            donate_argnums=donate,
            keep_unused=True,
        )
        self.sharding = NamedSharding(self.mesh, PartitionSpec("core"))

    def run(self, in_maps):
        np_ = np
        # concatenated inputs (axis 0 across cores)
        concat_in = [
            np_.concatenate([np_.asarray(in_maps[c][name]) for c in range(self.n_cores)],
                            axis=0)
            for name in self.in_names
        ]
        zeros = [np_.zeros((self.n_cores * z.shape[0], *z.shape[1:]), z.dtype)
                 for z in self.zero_outs]
        out = self.fn(*concat_in, *zeros)
        self.jax.block_until_ready(out)
        return [
            {n: np_.asarray(out[i]).reshape(self.n_cores, *self.out_avals[i].shape)[c]
             for i, n in enumerate(self.out_names)}
            for c in range(self.n_cores)
        ]


_CACHE = {}


def _get_runner(caps_key):
    if caps_key not in _CACHE:
        nc = _build_nc(list(caps_key))
        _CACHE[caps_key] = _SpmdRunner(nc, N_CORES)
    return _CACHE[caps_key]


def kernel(features: np.ndarray, neigh_idx: np.ndarray, seg_ids: np.ndarray,
           ) -> np.ndarray:
    features = np.ascontiguousarray(np.asarray(features, dtype=np.float32))
    neigh = np.asarray(neigh_idx).astype(np.int64)
    seg = np.asarray(seg_ids).astype(np.int64)
    assert features.shape == (N_NODES, D)
    assert neigh.shape == (N_EDGES,) and seg.shape == (N_EDGES,)

    # hi/lo bf16 split, packed side by side -> [N, 128] bf16
    hi = features.astype(ml_dtypes.bfloat16)
    lo = (features - hi.astype(np.float32)).astype(ml_dtypes.bfloat16)
    packed = np.ascontiguousarray(np.concatenate([hi, lo], axis=1))

    ebounds = np.searchsorted(seg, np.arange(N_CORES + 1) * SEGS_PER_CORE)
    caps = _compute_caps(neigh, seg, ebounds)
    runner = _get_runner(tuple(caps))

    iota_in = np.tile(np.arange(W, dtype=np.float32), (128, 1)).astype(ml_dtypes.bfloat16)
    tabs = {f"tab{k}": packed[k * BUCKET : min((k + 1) * BUCKET, N_NODES)]
            for k in range(NBUK)}
    in_maps = []
    for c in range(N_CORES):
        idx16, rel = _prep_core(neigh, seg, ebounds[c], ebounds[c + 1],
                                c * SEGS_PER_CORE, caps)
        m = dict(tabs)
        m.update({"idx16": idx16, "rel": rel, "iota": iota_in})
        in_maps.append(m)

    results = runner.run(in_maps)
    out = np.empty((N_NODES, D), np.float32)
    for c in range(N_CORES):
        out[c * SEGS_PER_CORE : (c + 1) * SEGS_PER_CORE] = \
            results[c]["out"][:SEGS_PER_CORE]
    return out


# revision 5
# speedup vs baseline: 81.5291x; 81.5291x over previous
"""Trainium2 Bass kernel for gather + segment-sum (GNN sum-aggregator).

    out[s, :] = sum_{e : seg_ids[e] == s} features[neigh_idx[e], :]

Strategy (8 NeuronCores, SPMD single NEFF):
  - Shard the segment (destination-node) axis: core c owns segments
    [12500c, 12500(c+1)) and the contiguous slice of the sorted edge list
    that targets them. The feature table is replicated.
  - Features are split hi/lo into two bf16 halves packed side by side
    ([N, 128] bf16), so one 256B-row gather fetches an exactly-representable
    fp32 row as two bf16 matmul operands (rel err ~1e-7 after the PSUM-fp32
    accumulate).
  - Edges are grouped into 128-segment windows. Within a window they are
    sorted by neigh_idx and split into <=4 buckets of 32768 table rows so the
    optimized int16-index SWDGE dma_gather can fetch them; the 4 gathers of a
    window round-robin over 4 SWDGE queues (4x descriptor-gen parallelism,
    ~290 GB/s measured vs ~34 GB/s on one queue).
  - Segment-sum inside a window = matmul with a per-block one-hot matrix
    (lhsT = onehot[128 edges, 128 segs] bf16, built on DVE via
    tensor_scalar(is_equal) against an iota row; rhs = gathered [128, 128]
    bf16), accumulated over the window's blocks in PSUM, then flushed as one
    contiguous 128-row store. Pad slots gather row 0 and carry a one-hot
    miss value, so they contribute exactly zero.
"""

import math

import numpy as np
import ml_dtypes

N_NODES = 100000
N_EDGES = 1600000
D = 64
N_CORES = 8
SEGS_PER_CORE = N_NODES // N_CORES  # 12500
W = 128  # segments per window
NWIN = math.ceil(SEGS_PER_CORE / W)  # 98
SEG_PAD = NWIN * W  # 12544
BUCKET = 32768
NBUK = 4
MISS = 30000.0


def _wrap_idxs(idx_flat: np.ndarray) -> np.ndarray:
    """[NI] -> [128, NI//16] int16 (16-partition wrap, replicated 8x)."""
    ni = idx_flat.shape[0]
    w = idx_flat.reshape(ni // 16, 16).T.astype(np.int16)
    return np.tile(w, (8, 1))


def _prep_core(neigh: np.ndarray, seg: np.ndarray, e0: int, e1: int, seg_base: int,
               caps: list[int]):
    """Build idx16/rel arrays for one core given its edge slice [e0, e1)."""
    spw = sum(caps)
    nblk = spw // 128
    idx16 = np.zeros((128, NWIN * (spw // 16)), np.int16)
    rel = np.full((128, NWIN * nblk), MISS, np.float32)
    nidx = neigh[e0:e1]
    nseg = seg[e0:e1]
    # window of each edge (local)
    win_bounds = np.searchsorted(nseg, seg_base + np.arange(NWIN + 1) * W)
    for w in range(NWIN):
        a, b = win_bounds[w], win_bounds[w + 1]
        widx = nidx[a:b]
        wrel = (nseg[a:b] - (seg_base + w * W)).astype(np.float32)
        order = np.argsort(widx, kind="stable")
        widx = widx[order]
        wrel = wrel[order]
        bb = np.searchsorted(widx, np.arange(1, NBUK) * BUCKET)
        runs = np.split(np.arange(len(widx)), bb)
        scol = w * (spw // 16)
        sblk = w * nblk
        for k in range(NBUK):
            cap = caps[k]
            r = runs[k]
            assert len(r) <= cap, f"bucket overflow {len(r)} > {cap}"
            idxp = np.zeros(cap, np.int64)
            idxp[: len(r)] = widx[r] - k * BUCKET
            relp = np.full(cap, MISS, np.float32)
            relp[: len(r)] = wrel[r]
            idx16[:, scol : scol + cap // 16] = _wrap_idxs(idxp)
            rel[:, sblk : sblk + cap // 128] = relp.reshape(cap // 128, 128).T
            scol += cap // 16
            sblk += cap // 128
    return idx16, rel


def _compute_caps(neigh: np.ndarray, seg: np.ndarray, ebounds: np.ndarray):
    """Global per-bucket slot capacities (multiples of 128) over all (core, window)."""
    maxes = [0] * NBUK
    for c in range(N_CORES):
        e0, e1 = ebounds[c], ebounds[c + 1]
        nseg = seg[e0:e1]
        nidx = neigh[e0:e1]
        seg_base = c * SEGS_PER_CORE
        win_bounds = np.searchsorted(nseg, seg_base + np.arange(NWIN + 1) * W)
        for w in range(NWIN):
            a, b = win_bounds[w], win_bounds[w + 1]
            widx = np.sort(nidx[a:b])
            bb = np.searchsorted(widx, np.arange(NBUK + 1) * BUCKET)
            for k in range(NBUK):
                maxes[k] = max(maxes[k], bb[k + 1] - bb[k])
    return [max(128, 128 * math.ceil(m / 128)) for m in maxes]


def _build_nc(caps: list[int], repeat: int = 1):
    import concourse.bass as bass
    import concourse.tile as tile
    from concourse import bacc, mybir

    spw = sum(caps)
    nblk = spw // 128
    tab_sizes = [min(BUCKET, N_NODES - k * BUCKET) for k in range(NBUK)]

    nc = bacc.Bacc("TRN2", target_bir_lowering=False, num_swdge_queues=4)
    tabs = [
        nc.dram_tensor(f"tab{k}", [tab_sizes[k], 2 * D], mybir.dt.bfloat16,
                       kind="ExternalInput")
        for k in range(NBUK)
    ]
    idx_d = nc.dram_tensor("idx16", [128, NWIN * (spw // 16)], mybir.dt.int16,
                           kind="ExternalInput")
    rel_d = nc.dram_tensor("rel", [128, NWIN * nblk], mybir.dt.float32,
                           kind="ExternalInput")
    iota_d = nc.dram_tensor("iota", [128, W], mybir.dt.bfloat16, kind="ExternalInput")
    out_d = nc.dram_tensor("out", [SEG_PAD, D], mybir.dt.float32, kind="ExternalOutput")

    with tile.TileContext(nc) as tc:
        with (
            tc.tile_pool(name="const", bufs=1) as cpool,
            tc.tile_pool(name="g", bufs=3) as gpool,
            tc.tile_pool(name="oh", bufs=8) as ohpool,
            tc.tile_pool(name="psum", bufs=4, space="PSUM") as ppool,
            tc.tile_pool(name="flush", bufs=4) as fpool,
        ):
            idx_t = cpool.tile([128, NWIN * (spw // 16)], mybir.dt.int16)
            nc.sync.dma_start(idx_t[:], idx_d[:])
            rel_t = cpool.tile([128, NWIN * nblk], mybir.dt.float32)
            nc.sync.dma_start(rel_t[:], rel_d[:])
            iota_t = cpool.tile([128, W], mybir.dt.bfloat16)
            nc.sync.dma_start(iota_t[:], iota_d[:])

            def window(w):
                g_t = gpool.tile([128, nblk, 2 * D], mybir.dt.bfloat16)
                scol = w * (spw // 16)
                sslot = 0
                for k in range(NBUK):
                    cap = caps[k]
                    nc.gpsimd.dma_gather(
                        g_t[:, sslot : sslot + cap // 128, :],
                        tabs[k][:],
                        idx_t[:, scol : scol + cap // 16],
                        cap, cap, 2 * D,
                        single_packet=False,
                        queue_num=k % 4,
                    )
                    scol += cap // 16
                    sslot += cap // 128
                psum_t = ppool.tile([W, 2 * D], mybir.dt.float32, space="PSUM")
                for b in range(nblk):
                    oh = ohpool.tile([128, W], mybir.dt.bfloat16)
                    nc.vector.tensor_scalar(
                        out=oh[:], in0=iota_t[:],
                        scalar1=rel_t[:, w * nblk + b : w * nblk + b + 1],
                        scalar2=None, op0=mybir.AluOpType.is_equal,
                    )
                    nc.tensor.matmul(
                        psum_t[:], lhsT=oh[:], rhs=g_t[:, b, :],
                        start=(b == 0), stop=(b == nblk - 1),
                    )
                cop = fpool.tile([W, 2 * D], mybir.dt.float32)
                nc.scalar.copy(cop[:], psum_t[:])
                comb = fpool.tile([W, D], mybir.dt.float32)
                nc.vector.tensor_tensor(
                    out=comb[:], in0=cop[:, :D], in1=cop[:, D:],
                    op=mybir.AluOpType.add,
                )
                nc.sync.dma_start(out_d[w * W : (w + 1) * W, :], comb[:])

            if repeat > 1:
                with tc.For_i(0, repeat, 1):
                    for w in range(NWIN):
                        window(w)
            else:
                for w in range(NWIN):
                    window(w)
    nc.finalize()
    return nc


class _SpmdRunner:
    """Compile once, execute the bass kernel across n_cores via PJRT shard_map."""

    def __init__(self, nc, n_cores: int):
        import jax
        import numpy as np
        from jax.experimental.shard_map import shard_map
        from jax.sharding import Mesh, NamedSharding, PartitionSpec
        import concourse.mybir as mybir
        from concourse.bass2jax import (
            _bass_exec_p, install_neuronx_cc_hook, partition_id_tensor,
        )

        install_neuronx_cc_hook()
        self.jax = jax
        self.n_cores = n_cores
        in_names, out_names, out_avals, zero_outs = [], [], [], []
        partition_name = nc.partition_id_tensor.name if nc.partition_id_tensor else None
        for alloc in nc.m.functions[0].allocations:
            if not isinstance(alloc, mybir.MemoryLocationSet):
                continue
            name = alloc.memorylocations[0].name
            if alloc.kind == "ExternalInput":
                if name != partition_name:
                    in_names.append(name)
            elif alloc.kind == "ExternalOutput":
                shape = tuple(alloc.tensor_shape)
                dtype = mybir.dt.np(alloc.dtype)
                out_names.append(name)
                out_avals.append(jax.core.ShapedArray(shape, dtype))
                zero_outs.append(np.zeros(shape, dtype))
        self.n_params = len(in_names)
        self.in_names = list(in_names)
        self.out_names = out_names
        self.out_avals = out_avals
        self.zero_outs = zero_outs
        all_in = in_names + out_names + ([partition_name] if partition_name else [])

        def _body(*args):
            operands = list(args)
            if partition_name is not None:
                operands.append(partition_id_tensor())
            outs = _bass_exec_p.bind(
                *operands,
                out_avals=tuple(out_avals),
                in_names=tuple(all_in),
                out_names=tuple(out_names),
                lowering_input_output_aliases=(),
                sim_require_finite=True,
                sim_require_nnan=True,
                nc=nc,
            )
            return tuple(outs)

        donate = tuple(range(self.n_params, self.n_params + len(out_names)))
        devices = jax.devices()[:n_cores]
        assert len(devices) >= n_cores, f"need {n_cores} cores, got {len(devices)}"
        self.mesh = Mesh(np.asarray(devices), ("core",))
        in_specs = (PartitionSpec("core"),) * (self.n_params + len(out_names))
        out_specs = (PartitionSpec("core"),) * len(out_names)
        self.fn = jax.jit(
            shard_map(_body, mesh=self.mesh, in_specs=in_specs, out_specs=out_specs,
                      check_rep=False),
            donate_argnums=donate,
            keep_unused=True,
        )
        self.sharding = NamedSharding(self.mesh, PartitionSpec("core"))

    def run(self, in_maps):
        np_ = np
        # concatenated inputs (axis 0 across cores)
        concat_in = [
            np_.concatenate([np_.asarray(in_maps[c][name]) for c in range(self.n_cores)],
                            axis=0)
            for name in self.in_names
        ]
        zeros = [np_.zeros((self.n_cores * z.shape[0], *z.shape[1:]), z.dtype)
                 for z in self.zero_outs]
        out = self.fn(*concat_in, *zeros)
        self.jax.block_until_ready(out)
        return [
            {n: np_.asarray(out[i]).reshape(self.n_cores, *self.out_avals[i].shape)[c]
             for i, n in enumerate(self.out_names)}
            for c in range(self.n_cores)
        ]


_CACHE = {}


def _get_runner(caps_key):
    if caps_key not in _CACHE:
        nc = _build_nc(list(caps_key))
        _CACHE[caps_key] = _SpmdRunner(nc, N_CORES)
    return _CACHE[caps_key]


def kernel(features: np.ndarray, neigh_idx: np.ndarray, seg_ids: np.ndarray,
           ) -> np.ndarray:
    features = np.ascontiguousarray(np.asarray(features, dtype=np.float32))
    neigh = np.asarray(neigh_idx).astype(np.int64)
    seg = np.asarray(seg_ids).astype(np.int64)
    assert features.shape == (N_NODES, D)
    assert neigh.shape == (N_EDGES,) and seg.shape == (N_EDGES,)

    # hi/lo bf16 split, packed side by side -> [N, 128] bf16
    hi = features.astype(ml_dtypes.bfloat16)
    lo = (features - hi.astype(np.float32)).astype(ml_dtypes.bfloat16)
    packed = np.ascontiguousarray(np.concatenate([hi, lo], axis=1))

    ebounds = np.searchsorted(seg, np.arange(N_CORES + 1) * SEGS_PER_CORE)
    caps = _compute_caps(neigh, seg, ebounds)
    runner = _get_runner(tuple(caps))

    iota_in = np.tile(np.arange(W, dtype=np.float32), (128, 1)).astype(ml_dtypes.bfloat16)
    tabs = {f"tab{k}": packed[k * BUCKET : min((k + 1) * BUCKET, N_NODES)]
            for k in range(NBUK)}
    in_maps = []
    for c in range(N_CORES):
        idx16, rel = _prep_core(neigh, seg, ebounds[c], ebounds[c + 1],
                                c * SEGS_PER_CORE, caps)
        m = dict(tabs)
        m.update({"idx16": idx16, "rel": rel, "iota": iota_in})
        in_maps.append(m)

    results = runner.run(in_maps)
    out = np.empty((N_NODES, D), np.float32)
    for c in range(N_CORES):
        out[c * SEGS_PER_CORE : (c + 1) * SEGS_PER_CORE] = \
            results[c]["out"][:SEGS_PER_CORE]
    return out
